# revision 6
# baseline (speedup 1.0000x reference)
"""Trainium2 Bass kernel for nn_Attention (dense transformer MHA block).

Reference computation (per batch element b of 8):
    qkv = x @ w_qkv;  q,k,v split into 16 heads of dim 64
    out = softmax(q k^T / 8) v  (per head),  y = out @ w_proj + b_proj

Sharding: pure data-parallel over the batch (B=8 == n_cores). Each core
computes one batch element's full attention with replicated weights; no
collectives. Full inputs in, full outputs out; gather = np.stack.

Per-core dataflow (fp32r for the dense GEMMs, bf16 for S/PV/proj scores,
fp32 PSUM accumulate everywhere):
  1. x is DMA'd per n-tile and PE-transposed (fp32) into xT [d, n]; the
     fp32r rounding happens in the PSUM->SBUF eviction copies (the BIR
     verifier requires fp32r operands to come from a rounding engine op,
     so weights get small DVE convert copies from f32 staging, batched
     into a few wide DMAs to amortize descriptor overhead).
  2. v-pass chains (xT-stationary, w_v-moving) run inside pairs 0-1 as
     PE filler; results land in v_aug [n, 16*(64+1)] bf16 with a ones
     column per head (integrated softmax denominator).
  3. per pair: qT,kT [c,n] bf16 via w-stationary / xT-moving chains.
     Both heads of the pair advance together through each j-tile: their
     S matmuls (K=64) are issued back-to-back as 64-row PE tiles
     (tile_position rows 0/64) so they run concurrently in the array;
     each S/P tile packs [head-even | head-odd] for one i-half, and the
     two i-half sweeps are serialized so PV only ever needs two PSUM
     accumulators.
  4. P = exp(S/8) on ScalarE (PSUM -> SBUF, bf16); PV chains accumulate
     outT[65, i] over j (v_aug-stationary, P-moving); row 64 is the
     softmax denominator. normalize: reciprocal (DVE) ->
     partition_broadcast (GpSimd) -> multiply (DVE) into attn [c,n] bf16.
  5. proj (bf16): attn-stationary @ w_proj-moving; the S pool's PSUM
     banks are recycled into a 4-buf proj pool the moment the last exp
     drains (so six chains pre-run their cc 0-6 matmuls under the final
     head's normalize), bias added during eviction (DVE), per-half y DMA.

  Scheduling notes: qT/kT and the first x tile are split into per-chunk
  tiles because dependency tracking is tile-granular; S matmuls for two
  j-tiles are batched back-to-back to halve PE array mode switches.
"""

import numpy as np
from contextlib import ExitStack

import concourse.bass as bass
import concourse.bacc as bacc
import concourse.mybir as mybir
from concourse import tile
from concourse.bass_utils import run_bass_kernel_spmd
from concourse.masks import make_identity

F32 = mybir.dt.float32
F32R = mybir.dt.float32r
BF16 = mybir.dt.bfloat16
EXPF = mybir.ActivationFunctionType.Exp

CONFIG = {
    "pv_bf16": True,   # v_aug + P in bf16 (PV matmul in bf16)
    "p_bufs": 16,
    "wqk_bufs": 2,
    "s_bufs": 2,
    "qk_sb_bufs": 4,
    "qk_bf16": True,
    "v_in_pairs": True,
    "rt_bufs": 2,
}

N = 1024          # sequence length (per core)
D = 1024          # model dim
H = 16            # heads
HD = 64           # head dim
SCALE = HD ** -0.5
P = 128           # partitions
NT = N // P       # 8 n-tiles
DT = D // P       # 8 d-chunks
NCORES = 8


def _wqk_dma(nc, pool, stage, wqkv_d, pair):
    """One DMA for the pair's q+k weight columns, one convert to f32r.

    Returns the [P, DT, 2, P] f32r tile; slice [:, dt, which, :] at use."""
    wf = stage.tile([P, DT, 2, P], F32, tag="wbig", name=f"wqkf{pair}")
    src = wqkv_d.rearrange("(dt p) (k r e) -> p dt k r e", p=P, k=3,
                           e=P)[:, :, 0:2, pair, :]
    for which in range(2):
        nc.sync.dma_start(wf[:, :, which, :], src[:, :, which, :])
    wr = pool.tile([P, DT, 2, P], F32R, tag="wqk", name=f"wqk{pair}")
    nc.vector.tensor_copy(wr[:], wf[:])
    return wr


def _build(tc, nc, x_d, wqkv_d, wproj_d, bproj_d, y_d):
    mul = mybir.AluOpType.mult
    add = mybir.AluOpType.add
    VD = BF16 if CONFIG["pv_bf16"] else F32
    _v = (lambda ap: ap) if CONFIG["pv_bf16"] else _r
    QD = BF16 if CONFIG["qk_bf16"] else F32
    _q = (lambda ap: ap) if CONFIG["qk_bf16"] else _r

    with ExitStack() as outer:
        const = outer.enter_context(tc.tile_pool(name="const", bufs=1))
        ident = const.tile([P, P], F32)
        ones16 = const.tile([P, H], F32)
        bias_bc = const.tile([P, D], F32)

        xt_pool = outer.enter_context(tc.tile_pool(name="xT", bufs=2 * DT))
        # two half-tiles per d-chunk: n 0-511 and 512-1023. Dep tracking
        # is tile-granular, so qk chains for one n-half only wait on that
        # half's transposes (4 n-tiles) instead of all 8.
        xT = [(xt_pool.tile([P, N // 2], F32R, tag="xT", name=f"xTa{i}"),
               xt_pool.tile([P, N // 2], F32R, tag="xT", name=f"xTb{i}"))
              for i in range(DT)]

        def xT_n(dt, nt):
            return xT[dt][nt // 4][:, (nt % 4) * P:(nt % 4 + 1) * P]
        vaug_pool = outer.enter_context(tc.tile_pool(name="vaug", bufs=NT))
        vaug = [vaug_pool.tile([P, H * (HD + 1)], VD, tag="vaug",
                               name=f"vaug{i}") for i in range(NT)]
        attn_pool = outer.enter_context(tc.tile_pool(name="attnout", bufs=DT))
        attn_t = [attn_pool.tile([P, N], BF16, tag="attn", name=f"attn{i}")
                  for i in range(DT)]
        outsb = outer.enter_context(tc.tile_pool(name="outsb", bufs=3))
        mm_ps = outer.enter_context(
            tc.tile_pool(name="mmps", bufs=2, space="PSUM"))
        wqk_pool = outer.enter_context(
            tc.tile_pool(name="wqk", bufs=CONFIG["wqk_bufs"]))
        wv_pool = outer.enter_context(tc.tile_pool(name="wv", bufs=4))
        wstage = outer.enter_context(tc.tile_pool(name="wstage", bufs=2))

        # ---- phase A: x load + transpose (v-pass happens in pairs 0-1) ----
        with ExitStack() as phA:
            xload = phA.enter_context(tc.tile_pool(name="xload", bufs=4))
            tp_ps = phA.enter_context(
                tc.tile_pool(name="tpps", bufs=4, space="PSUM"))

            # x DMAs first: the first transpose gates PE start. Tile 0 is
            # split so PE can start after a 256-column sliver lands.
            xf0a = xload.tile([P, 256], F32, tag="xf0a", name="xf0a")
            nc.sync.dma_start(xf0a[:], x_d[0:P, 0:256])
            xf0b = xload.tile([P, D - 256], F32, tag="xf0b", name="xf0b")
            nc.sync.dma_start(xf0b[:], x_d[0:P, 256:D])
            xf = [(xf0a, xf0b)]
            for nt in range(1, NT):
                xt_ = xload.tile([P, D], F32, tag="xf", name=f"xf{nt}")
                nc.sync.dma_start(xt_[:], x_d[nt * P:(nt + 1) * P, :])
                xf.append(xt_)

            def xf_chunk(nt, dt):
                if nt == 0:
                    return (xf0a[:, dt * P:(dt + 1) * P] if dt < 2
                            else xf0b[:, (dt - 2) * P:(dt - 1) * P])
                return xf[nt][:, dt * P:(dt + 1) * P]

            # weight DMA order matches PE stream consumption: qk pair 0,
            # then v weights cv0 (heads 0-7), qk pair 1, v weights cv1
            wqk_tiles = {}
            wv_r = {}
            wsrc = wqkv_d.rearrange("(dt p) e -> p dt e", p=P)
            for grp in range(2):
                wqk_tiles[grp] = _wqk_dma(nc, wqk_pool, wstage, wqkv_d, grp)
                for half in range(2):
                    wvt = wv_pool.tile([P, 4, 512], F32R, tag="wv",
                                       name=f"wv{grp}h{half}")
                    wf = wstage.tile([P, 4, 512], F32, tag="wbig",
                                     name=f"wvf{grp}_{half}")
                    nc.sync.dma_start(
                        wf[:],
                        wsrc[:, half * 4:(half + 1) * 4,
                             2 * D + grp * 512: 2 * D + (grp + 1) * 512])
                    nc.vector.tensor_copy(wvt[:], wf[:])
                    for dt in range(4):
                        wv_r[(grp, half * 4 + dt)] = wvt[:, dt, :]

            make_identity(nc, ident[:])
            nc.gpsimd.memset(ones16[:], 1.0)
            bst_pool = phA.enter_context(tc.tile_pool(name="bst", bufs=1))
            bstage = bst_pool.tile([1, D], F32)
            nc.sync.dma_start(bstage[:],
                              bproj_d[:].rearrange("(a f) -> a f", a=1))
            nc.gpsimd.partition_broadcast(bias_bc[:], bstage[:])

            def v_chains(cv, nt_list):
                for nt in nt_list:
                    vp = mm_ps.tile([P, 512], F32, tag="mm")
                    for dt in range(DT):
                        nc.tensor.matmul(
                            vp[:], xT_n(dt, nt),
                            wv_r[(cv, dt)], start=(dt == 0),
                            stop=(dt == DT - 1))
                    dst = vaug[nt][:].rearrange(
                        "p (h e) -> p h e", h=H)[:, 8 * cv:8 * cv + 8, 0:HD]
                    src = vp[:].rearrange("p (h e) -> p h e", h=8)
                    nc.vector.tensor_copy(dst, src)

            for nt in range(NT):
                for dt in range(DT):
                    tp = tp_ps.tile([P, P], F32, tag="tp")
                    nc.tensor.transpose(tp[:], xf_chunk(nt, dt), ident[:])
                    nc.vector.tensor_copy(xT_n(dt, nt), tp[:])
                nc.vector.tensor_copy(
                    vaug[nt][:].rearrange("p (h e) -> p h e", h=H)[:, :, HD:HD + 1],
                    ones16[:].rearrange("p (h e) -> p h e", e=1))
                if not CONFIG["v_in_pairs"]:
                    v_chains(0, [nt])
                    v_chains(1, [nt])

        # ---- phase B: per head pair qk + attention; wp prefetch ----
        with ExitStack() as phB:
            wp_pool = phB.enter_context(tc.tile_pool(name="wp", bufs=1))
            wp_t = wp_pool.tile([P, DT, D], BF16, tag="wp", name="wp_t")
            with ExitStack() as pairs_st:
                qk_pool = pairs_st.enter_context(
                    tc.tile_pool(name="qk", bufs=CONFIG["qk_sb_bufs"]))
                p_pool = pairs_st.enter_context(
                    tc.tile_pool(name="pT", bufs=CONFIG["p_bufs"]))
                pv_ps = pairs_st.enter_context(
                    tc.tile_pool(name="pvps", bufs=2, space="PSUM"))
                rt_pool = pairs_st.enter_context(
                    tc.tile_pool(name="rt", bufs=CONFIG["rt_bufs"]))
                bt_pool = pairs_st.enter_context(
                    tc.tile_pool(name="bt", bufs=CONFIG["rt_bufs"]))

                s_st = ExitStack()
                s_ps = s_st.enter_context(
                    tc.tile_pool(name="sps", bufs=CONFIG["s_bufs"],
                                 space="PSUM"))
                for pair in range(H // 2):
                    if pair + 2 <= H // 2 - 1:
                        wqk_tiles[pair + 2] = _wqk_dma(
                            nc, wqk_pool, wstage, wqkv_d, pair + 2)
                    if pair >= 4:
                        # prefetch w_proj (bf16): 2 cc-chunks per late pair
                        cc0 = 2 * (pair - 4)
                        wf = wstage.tile([P, 2, D], F32, tag="wbig",
                                         name=f"wpf{pair}")
                        nc.sync.dma_start(
                            wf[:],
                            wproj_d.rearrange("(cc p) e -> p cc e",
                                              p=P)[:, cc0:cc0 + 2, :])
                        nc.vector.tensor_copy(wp_t[:, cc0:cc0 + 2, :], wf[:])
                    wqk_r = wqk_tiles.pop(pair)
                    # per-(which, nch) tiles: tile-granular dep tracking
                    # then lets S matmuls start after half the qk chains
                    qk_t = {}
                    for which in range(2):   # 0 = q, 1 = k
                        for nch in range(2):
                            ct = qk_pool.tile([P, 512], QD, tag="qk",
                                              name=f"qk{pair}_{which}{nch}")
                            qp = mm_ps.tile([P, 512], F32, tag="mm")
                            for dt in range(DT):
                                nc.tensor.matmul(
                                    qp[:], wqk_r[:, dt, which, :],
                                    xT[dt][nch][:],
                                    start=(dt == 0), stop=(dt == DT - 1))
                            nc.vector.tensor_copy(ct[:], qp[:])
                            qk_t[(which, nch)] = ct

                    if CONFIG["v_in_pairs"] and pair == 0:
                        # cv0 feeds heads 0-7: must precede pair 0's heads
                        # (PV h0 consumes vaug[jt] immediately)
                        v_chains(0, range(NT))
                    # Both heads of the pair advance together: their S
                    # matmuls are 64-row PE tiles (rows 0-63 / 64-127) issued
                    # back-to-back, so they run concurrently in the array.
                    # Each s/p tile packs [head-even | head-odd] for one ich
                    # half; the two ich sweeps are serialized so PV only
                    # needs two PSUM accumulators at a time.
                    h_e, h_o = 2 * pair, 2 * pair + 1
                    for ich in range(2):
                        sl = slice(ich * 512, (ich + 1) * 512)
                        pv_e = pv_ps.tile([HD + 1, 512], F32, tag="pv",
                                          name=f"pv{h_e}_{ich}")
                        pv_o = pv_ps.tile([HD + 1, 512], F32, tag="pv",
                                          name=f"pv{h_o}_{ich}")
                        for jtb in range(0, NT, 2):
                            sps, pts = [], []
                            for jt in (jtb, jtb + 1):
                                sp = s_ps.tile([P, N], F32, tag="s")
                                kt_ = qk_t[(1, jt // 4)]
                                kc = (jt % 4) * P
                                qt_ = qk_t[(0, ich)]
                                nc.tensor.matmul(
                                    sp[:, 0:512],
                                    _q(kt_[0:HD, kc:kc + P]),
                                    _q(qt_[0:HD, :]),
                                    start=True, stop=True,
                                    tile_position=(0, 0))
                                nc.tensor.matmul(
                                    sp[:, 512:1024],
                                    _q(kt_[HD:P, kc:kc + P]),
                                    _q(qt_[HD:P, :]),
                                    start=True, stop=True,
                                    tile_position=(64, 0))
                                sps.append(sp)
                            for jt in (jtb, jtb + 1):
                                pt = p_pool.tile([P, N], VD, tag="p",
                                                 name=f"pT{pair}_{ich}_{jt}")
                                nc.scalar.activation(pt[:], sps[jt - jtb][:],
                                                     EXPF, scale=SCALE)
                                pts.append(pt)
                            for jt in (jtb, jtb + 1):
                                pt = pts[jt - jtb]
                                nc.tensor.matmul(
                                    pv_e[:],
                                    _v(vaug[jt][:, h_e * (HD + 1):
                                                (h_e + 1) * (HD + 1)]),
                                    _v(pt[:, 0:512]),
                                    start=(jt == 0), stop=(jt == NT - 1))
                                nc.tensor.matmul(
                                    pv_o[:],
                                    _v(vaug[jt][:, h_o * (HD + 1):
                                                (h_o + 1) * (HD + 1)]),
                                    _v(pt[:, 512:1024]),
                                    start=(jt == 0), stop=(jt == NT - 1))
                        for hh, pvt in ((0, pv_e), (1, pv_o)):
                            base = HD * hh
                            rt = rt_pool.tile([1, 512], F32, tag="rt",
                                              name=f"rt{pair}_{hh}_{ich}")
                            bt = bt_pool.tile([HD, 512], F32, tag="bt",
                                              name=f"bt{pair}_{hh}_{ich}")
                            nc.vector.reciprocal(rt[:], pvt[HD:HD + 1, :])
                            nc.gpsimd.partition_broadcast(bt[:], rt[:])
                            nc.vector.tensor_tensor(
                                attn_t[pair][base:base + HD, sl],
                                pvt[0:HD, :], bt[:], mul)

                    if CONFIG["v_in_pairs"] and pair == 1:
                        # cv1 feeds heads 8-15 (pairs 4-7): post-heads
                        # PE filler under pair 1's exp stream
                        v_chains(1, range(NT))
                # ---- projection: the S pool's 4 banks free once the
                # last exp has read them (not the last normalize), so six
                # chains pre-run their cc 0-6 matmuls under the final
                # head's exp/normalize drain ----
                s_st.close()
                pj_ps = pairs_st.enter_context(
                    tc.tile_pool(name="pjps", bufs=4, space="PSUM"))
                for nt in range(NT):
                    yo = outsb.tile([P, D], F32, tag="y", name=f"yo{nt}")
                    for ec in range(2):
                        pool_ = mm_ps if (2 * nt + ec) % 3 == 2 else pj_ps
                        yp = pool_.tile([P, 512], F32,
                                        tag="mm" if pool_ is mm_ps else "pj")
                        for cc in range(DT):
                            nc.tensor.matmul(
                                yp[:], attn_t[cc][:, nt * P:(nt + 1) * P],
                                wp_t[:, cc, ec * 512:(ec + 1) * 512],
                                start=(cc == 0), stop=(cc == DT - 1))
                        nc.vector.tensor_tensor(
                            yo[:, ec * 512:(ec + 1) * 512], yp[:],
                            bias_bc[:, ec * 512:(ec + 1) * 512], add)
                        nc.sync.dma_start(
                            y_d[nt * P:(nt + 1) * P,
                                ec * 512:(ec + 1) * 512],
                            yo[:, ec * 512:(ec + 1) * 512])


def build_nc():
    nc = bacc.Bacc("TRN2", target_bir_lowering=False, debug=False)
    x_d = nc.dram_tensor("x", [N, D], F32, kind="ExternalInput").ap()
    wqkv_d = nc.dram_tensor("w_qkv", [D, 3 * D], F32, kind="ExternalInput").ap()
    wproj_d = nc.dram_tensor("w_proj", [D, D], F32, kind="ExternalInput").ap()
    bproj_d = nc.dram_tensor("b_proj", [D], F32, kind="ExternalInput").ap()
    y_d = nc.dram_tensor("y", [N, D], F32, kind="ExternalOutput").ap()
    with tile.TileContext(nc) as tc:
        _build(tc, nc, x_d, wqkv_d, wproj_d, bproj_d, y_d)
    nc.compile()
    return nc


_NC = None


def kernel(x, w_qkv, w_proj, b_proj):
    global _NC
    if _NC is None:
        _NC = build_nc()
    x = np.ascontiguousarray(np.asarray(x, dtype=np.float32))
    w_qkv = np.ascontiguousarray(np.asarray(w_qkv, dtype=np.float32))
    w_proj = np.ascontiguousarray(np.asarray(w_proj, dtype=np.float32))
    b_proj = np.ascontiguousarray(np.asarray(b_proj, dtype=np.float32))
    in_maps = [
        {"x": x[c], "w_qkv": w_qkv, "w_proj": w_proj, "b_proj": b_proj}
        for c in range(NCORES)
    ]
    res = run_bass_kernel_spmd(_NC, in_maps, list(range(NCORES)))
    return np.stack([res.results[c]["y"] for c in range(NCORES)], axis=0)
